# revision 18
# baseline (speedup 1.0000x reference)
"""Trainium2 Bass kernel for DemoGraphNet (2-layer GCN + mean-pool + MLP head).

Self-contained: hardcodes problem shapes and the 8-core sharding strategy.

Strategy (v2)
-------------
Nodes are partitioned contiguously across 8 cores (12500 each), then each
core's nodes are PERMUTED into 100 windows of 128 dst slots by a vector
bin-packing that balances per-(window, src-block) edge counts to <=512, so the
128-aligned bucket quotas carry only ~5% padding (the SWDGE gather feed is the
machine bottleneck at ~2.9ns/row, so padded rows are pure loss).

Self-loop edges are NOT materialized: the self contribution
inv^2[n]*(h@W)[n] is accumulated into each window PSUM with a PE transpose of
the own-table window (reloaded from DRAM on the idle HWDGE queues).

    table[n]  = inv_sqrt[n] * (h @ W)[n]       (bf16, built shard-wise)
    S[e, j]   = (dst_e == j)                   (one-hot, DVE iota-compare)
    agg[c, j] = sum_e table[src_e][c]*S[e, j] + (T_w)^T   (TensorE, PSUM)
    h_next    = relu(agg * inv[dst] + b)

The node tables are exchanged with CHUNKED AllGathers (4 chunks of 25 windows
per layer) so communication overlaps compute: gather block b only depends on
AllGather chunk b.  Edges are bucketed (dst-window, src-block-of-25600-rows)
with quotas maxed across cores (single SPMD program).  The gather stream is
block-major per supertile (contiguous idx per dma_gather) while the one-hot /
matmul processing stream is window-major (one DVE is_equal builds all of a
window's one-hots across blocks).  Mean-pooling is a batch-id one-hot matmul
fused into layer-2 window evacuation; per-graph sums are AllReduced and the
tiny MLP head runs replicated on every core.
"""

import math
import os
import sys

sys.path.insert(0, "/opt/trn_rl_repo")

import numpy as np
import ml_dtypes

import concourse.bass as bass
import concourse.mybir as mybir
import concourse.tile as tile
import concourse.bacc as bacc
from concourse import library_config
from concourse.bass_utils import run_bass_kernel_spmd

BF16 = ml_dtypes.bfloat16
F32 = np.float32


class Cfg:
    def __init__(self):
        self.N = 100000
        self.G = 256
        self.C = 8
        self.HID = 128
        self.OUT = 8
        self.NPC = self.N // self.C       # 12500
        self.NWC = 4                      # src blocks == AllGather chunks
        self.RPC = self.NPC // self.NWC   # 3125 real nodes per chunk
        self.WPC = 25                     # windows per chunk
        self.NW = self.NWC * self.WPC     # 100
        self.WIN = 128
        self.PAD_NPC = self.NW * self.WIN     # 12800
        self.SCHUNK = self.WPC * self.WIN     # 3200 rows per (core, chunk)
        self.BLKROWS = self.C * self.SCHUNK   # 25600 rows per block
        self.NBLK = self.NWC
        self.TBL_N = self.NBLK * self.BLKROWS  # 102400
        self.ST_W = 4
        self.NST = self.NW // self.ST_W   # 25
        self.CAP = 512                    # bin-packing target per bucket
        self.GLMAX = 20                   # max chunks per window-group


CFG = Cfg()


# ----------------------------------------------------------------- host prep
def _pack_windows(node_prof, wpc, nblk, win, cap):
    """Pack nodes into `wpc` windows (<=win nodes each), balancing per-block
    in-edge counts to <=cap (best effort). Returns window index per node."""
    n = node_prof.shape[0]
    loads = np.zeros((wpc, nblk), dtype=np.int64)
    counts = np.zeros(wpc, dtype=np.int64)
    win_of = np.empty(n, dtype=np.int64)
    order = np.argsort(-node_prof.sum(1), kind="stable")
    big = 1 << 40
    for i in order:
        p = node_prof[i]
        cand = loads + p
        score = cand.max(1) * 1000 + counts
        feas = (cand <= cap).all(1) & (counts < win)
        if feas.any():
            score = np.where(feas, score, big)
        else:
            score = np.where(counts < win, score, big)
        b = int(np.argmin(score))
        win_of[i] = b
        loads[b] += p
        counts[b] += 1
    # repair pass: move nodes out of overflowing (window, block) cells into
    # windows with headroom so every cell lands <= cap (quota exactly 512)
    for _ in range(200):
        over = np.argwhere(loads > cap)
        if len(over) == 0:
            break
        w, b = over[0]
        members = np.nonzero((win_of == w) & (node_prof[:, b] > 0))[0]
        members = members[np.argsort(-node_prof[members, b], kind="stable")]
        moved = False
        for i in members:
            p = node_prof[i]
            cand = loads + p
            feas = (cand <= cap).all(1) & (counts < win)
            feas[w] = False
            if feas.any():
                dst = int(np.argmin(np.where(feas, cand.max(1), big)))
                win_of[i] = dst
                loads[w] -= p
                counts[w] -= 1
                loads[dst] += p
                counts[dst] += 1
                moved = True
                if loads[w, b] <= cap:
                    break
        if not moved:
            break
    return win_of


def _host_prep(cfg, x, edge_index, batch):
    """Shard + marshal inputs. Index bookkeeping (sorting, packing, bucketing)
    plus layout; feature FLOPs all happen on device."""
    N, C, NPC = cfg.N, cfg.C, cfg.NPC
    NW, WIN, WPC, NWC = cfg.NW, cfg.WIN, cfg.WPC, cfg.NWC
    NBLK, BLKROWS, SCHUNK, RPC = cfg.NBLK, cfg.BLKROWS, cfg.SCHUNK, cfg.RPC
    ST_W, NST, PAD_NPC = cfg.ST_W, cfg.NST, cfg.PAD_NPC

    src = np.asarray(edge_index[0], dtype=np.int64)
    dst = np.asarray(edge_index[1], dtype=np.int64)
    batch = np.asarray(batch, dtype=np.int64)
    x = np.asarray(x, dtype=np.float32)

    deg = (np.bincount(dst, minlength=N) + 1).astype(np.float64)
    inv_sqrt = (1.0 / np.sqrt(deg)).astype(np.float32)

    core_of = np.arange(N) // NPC
    loc_of = np.arange(N) % NPC
    chunk_of = loc_of // RPC
    src_blk = (src % NPC) // RPC
    prof = np.zeros((N, NBLK), dtype=np.int64)
    np.add.at(prof, (dst, src_blk), 1)

    win_of = np.zeros(N, dtype=np.int64)
    slot_of = np.zeros(N, dtype=np.int64)
    for c in range(C):
        for k in range(NWC):
            sel = np.nonzero((core_of == c) & (chunk_of == k))[0]
            w_loc = _pack_windows(prof[sel], WPC, NBLK, WIN, cfg.CAP)
            win_of[sel] = k * WPC + w_loc
            for w in range(WPC):
                m = sel[w_loc == w]
                slot_of[m] = np.arange(len(m))

    prow = win_of * WIN + slot_of                  # permuted local padded row
    tblrow = chunk_of * BLKROWS + core_of * SCHUNK + (prow % SCHUNK)

    # bucket edges per (dst core, dst window, src block)
    e_core = core_of[dst]
    e_w = win_of[dst]
    e_blk = src_blk
    key = (e_core * NW + e_w) * NBLK + e_blk
    counts = np.bincount(key, minlength=C * NW * NBLK).reshape(C, NW, NBLK)
    quota = counts.max(axis=0)
    quota = ((quota + 127) // 128) * 128 * (quota > 0)

    # gather stream layout: st-major, block-major, window within (st, blk)
    bucket_base = np.zeros((NW, NBLK), dtype=np.int64)
    seg_off = np.zeros((NST, NBLK), dtype=np.int64)
    seg_len = np.zeros((NST, NBLK), dtype=np.int64)
    pos = 0
    for st in range(NST):
        for b in range(NBLK):
            seg_off[st, b] = pos
            for w in range(st * ST_W, (st + 1) * ST_W):
                bucket_base[w, b] = pos
                pos += quota[w, b]
            seg_len[st, b] = pos - seg_off[st, b]
    EP = pos
    NCH = EP // 128

    # processing order: st-major, window-major, block within window
    proc = []       # per st: [(w, [(b, tile_col), ...]), ...]
    proc_map = []   # flat [(st, w, b, col)]
    for st in range(NST):
        ents = []
        for w in range(st * ST_W, (st + 1) * ST_W):
            lst = []
            for b in range(NBLK):
                if quota[w, b] == 0:
                    continue
                col0 = (bucket_base[w, b] - seg_off[st, b]) // 128
                for cc in range(quota[w, b] // 128):
                    lst.append((b, col0 + cc))
                    proc_map.append((st, w, b, col0 + cc))
            ents.append((w, lst))
        proc.append(ents)
    assert len(proc_map) == NCH
    glmax = max(len(lst) for ents in proc for (_, lst) in ents)
    assert glmax <= cfg.GLMAX, glmax

    cnt = np.bincount(batch, minlength=cfg.G).astype(np.float32)

    in_maps = []
    for c in range(C):
        sel = np.nonzero(e_core == c)[0]
        bkey = e_w[sel] * NBLK + e_blk[sel]
        order = np.lexsort((tblrow[src[sel]], bkey))
        sel = sel[order]
        bkey = bkey[order]
        change = np.ones(len(sel), dtype=bool)
        change[1:] = bkey[1:] != bkey[:-1]
        gstart = np.maximum.accumulate(np.where(change, np.arange(len(sel)), 0))
        rank = np.arange(len(sel)) - gstart
        posn = bucket_base[e_w[sel], e_blk[sel]] + rank

        e_idx16 = np.zeros(EP, dtype=np.int16)
        e_idx16[posn] = (tblrow[src[sel]] - e_blk[sel] * BLKROWS).astype(np.int16)
        e_dst = np.full(EP, -1.0, dtype=np.float32)
        e_dst[posn] = (prow[dst[sel]] % WIN).astype(np.float32)

        idx_img = np.tile(e_idx16.reshape(-1, 16).T, (8, 1)).copy()
        # dst one-hot source values in PROCESSING order
        dst_col = np.empty((NCH, 128), dtype=np.float32)
        for j, (st, w, b, col) in enumerate(proc_map):
            g0 = int(seg_off[st, b]) + col * 128
            dst_col[j] = e_dst[g0:g0 + 128]
        dst_col = dst_col.T.astype(BF16).copy()

        lo = c * NPC
        own = np.arange(lo, lo + NPC)
        x_pad = np.zeros((PAD_NPC, cfg.HID), dtype=np.float32)
        x_pad[prow[own]] = x[lo:lo + NPC]
        xc = np.ascontiguousarray(
            x_pad.reshape(NW, 128, cfg.HID).transpose(2, 0, 1)).astype(BF16)

        inv_own = np.ones(PAD_NPC, dtype=np.float32)
        inv_own[prow[own]] = inv_sqrt[lo:lo + NPC]
        inv_pp = inv_own.reshape(NW, 128).T.copy()
        inv_b = np.broadcast_to(inv_own, (128, PAD_NPC)).astype(BF16).copy()

        batch_own = np.full(PAD_NPC, -1.0, dtype=np.float32)
        batch_own[prow[own]] = batch[lo:lo + NPC].astype(np.float32)
        batch_pp = batch_own.reshape(NW, 128).T.copy()

        in_maps.append({
            "xc": xc, "idx": idx_img, "dstc": dst_col,
            "invpp": inv_pp, "invb": inv_b, "batchpp": batch_pp,
            "iota128": np.broadcast_to(
                np.tile(np.arange(128, dtype=np.float32), cfg.GLMAX),
                (128, cfg.GLMAX * 128)).astype(BF16).copy(),
            "iotag": np.broadcast_to(
                np.arange(cfg.G, dtype=np.float32), (128, cfg.G)).astype(BF16).copy(),
            "cnt": cnt[None, :].copy(),
            "ones1": np.ones((1, 128), dtype=np.float32),
        })

    meta = dict(EP=EP, NCH=NCH, seg_off=seg_off, seg_len=seg_len, proc=proc)
    return in_maps, meta


def _add_weights(cfg, in_maps, W1, b1, W2, b2, Wh1, bh1, Wh2, bh2):
    wts = {
        "W1": np.asarray(W1, F32).astype(BF16),
        "b1": np.asarray(b1, F32).reshape(-1, 1),
        "W2": np.asarray(W2, F32).astype(BF16),
        "b2": np.asarray(b2, F32).reshape(-1, 1),
        "Wh1": np.asarray(Wh1, F32), "bh1": np.asarray(bh1, F32).reshape(-1, 1),
        "Wh2": np.asarray(Wh2, F32), "bh2": np.asarray(bh2, F32).reshape(-1, 1),
    }
    for m in in_maps:
        m.update(wts)


# ------------------------------------------------------------- program build
def _build(cfg, meta):
    NW, ST_W, NST, NBLK, WPC = cfg.NW, cfg.ST_W, cfg.NST, cfg.NBLK, cfg.WPC
    NCH, EP = meta["NCH"], meta["EP"]
    HID, G = cfg.HID, cfg.G
    bf = mybir.dt.bfloat16
    f32 = mybir.dt.float32

    no_coll = os.environ.get("GNN_NO_COLL") == "1"
    no_gather = os.environ.get("GNN_NO_GATHER") == "1"
    nc = bacc.Bacc("TRN2", target_bir_lowering=False, debug=False,
                   num_devices=cfg.C, num_swdge_queues=4)
    P = {}
    def param(name, shape, dt=f32):
        P[name] = nc.declare_dram_parameter(name, list(shape), dt, isOutput=False)
        return P[name]

    param("xc", [128, NW, 128], bf)
    param("idx", [128, EP // 16], mybir.dt.int16)
    param("dstc", [128, NCH], bf)
    param("invpp", [128, NW]); param("invb", [128, cfg.PAD_NPC], bf)
    param("batchpp", [128, NW])
    param("iota128", [128, cfg.GLMAX * 128], bf); param("iotag", [128, G], bf)
    param("cnt", [1, G]); param("ones1", [1, 128])
    param("W1", [HID, HID], bf); param("b1", [HID, 1])
    param("W2", [HID, HID], bf); param("b2", [HID, 1])
    param("Wh1", [HID, HID]); param("bh1", [HID, 1])
    param("Wh2", [HID, cfg.OUT]); param("bh2", [cfg.OUT, 1])
    t_out = nc.declare_dram_parameter("out", [cfg.OUT, G], f32, isOutput=True)

    replica = [list(range(cfg.C))]

    from concourse.masks import make_identity

    with tile.TileContext(nc) as tc:
        with (
            tc.tile_pool(name="const", bufs=1) as cp,
            tc.tile_pool(name="xchunk", bufs=2) as xp,
            tc.tile_pool(name="mtiles", bufs=10) as mp,
            tc.tile_pool(name="stiles", bufs=3) as sp,
            tc.tile_pool(name="evac", bufs=6) as ep,
            tc.tile_pool(name="tchunk", bufs=8) as fp,
            tc.tile_pool(name="psw", bufs=4, space="PSUM") as psw,
            tc.tile_pool(name="psa", bufs=3, space="PSUM") as psa,
            tc.tile_pool(name="psg", bufs=1, space="PSUM") as psg,
            tc.tile_pool(name="dram", bufs=1, space="DRAM") as dp,
        ):
            nc.gpsimd.load_library(library_config.mlp)

            def load(name, shape, dt=f32, eng=None):
                t = cp.tile(list(shape), dt, tag=f"c_{name}", name=f"c_{name}")
                (eng or nc.sync).dma_start(t[:], P[name][:])
                return t

            # small consts + weights first (phase A wants W1 immediately)
            W1_sb = load("W1", [HID, HID], bf); b1_sb = load("b1", [HID, 1])
            W2_sb = load("W2", [HID, HID], bf); b2_sb = load("b2", [HID, 1])
            Wh1_sb = load("Wh1", [HID, HID]); bh1_sb = load("bh1", [HID, 1])
            Wh2_sb = load("Wh2", [HID, cfg.OUT]); bh2_sb = load("bh2", [cfg.OUT, 1])
            invs_sb = load("invpp", [128, NW])
            batch_sb = load("batchpp", [128, NW])
            iotag_sb = load("iotag", [128, G], bf)
            cnt_sb = load("cnt", [1, G])
            ones1_sb = load("ones1", [1, 128])

            # big consts on the sync queue; phase A's xc loads and table
            # chunk writes ride the scalar-engine queue so neither blocks
            # the other (the tile scheduler orders per-queue by readiness).
            idx_sb = load("idx", [128, EP // 16], mybir.dt.int16)
            dst_sb = load("dstc", [128, NCH], bf)
            invb_sb = load("invb", [128, cfg.PAD_NPC], bf)
            iota_sb = load("iota128", [128, cfg.GLMAX * 128], bf)

            ident = cp.tile([128, 128], f32, tag="c_ident")
            make_identity(nc, ident[:])
            identb = cp.tile([128, 128], bf, tag="c_identb")
            nc.vector.tensor_copy(out=identb[:], in_=ident[:])

            # node tables: per-chunk own tensors + per-block gathered tensors.
            # tbl_full registered in the DGE table (SWDGE descriptor
            # relocation requirement; Tile's symbolic lowering drops it).
            tbl_own = [[nc.dram_tensor(f"tblown{l}_{k}", [cfg.SCHUNK, HID], bf)
                        for k in range(cfg.NWC)] for l in range(2)]
            tbl_full = [[nc.dram_tensor(f"tblfull{l}_{b}", [cfg.BLKROWS, HID], bf,
                                     addr_space="Shared")
                         for b in range(NBLK)] for l in range(2)]
            for l in range(2):
                for t in tbl_full[l]:
                    mloc = nc.lookup_mloc(t)
                    if mloc.table_entry_id is None:
                        mloc.table_entry_id = len(nc.dge_table) + 1
                        nc.dge_table.append(mloc.name)

            def all_gather(layer, k):
                if no_coll:
                    nc.sync.dma_start(tbl_full[layer][k][:cfg.SCHUNK, :],
                                      tbl_own[layer][k][:])
                else:
                    nc.gpsimd.collective_compute(
                        "AllGather", mybir.AluOpType.bypass,
                        ins=[tbl_own[layer][k][:]], outs=[tbl_full[layer][k][:]],
                        replica_groups=replica)

            # ---- phase A (layer 1): table1 = invs * (x @ W1), node-major.
            # Windows accumulate into a resident SBUF chunk tile ([node, w, ch]
            # for 25 windows); one DMA ships the chunk to DRAM for AllGather,
            # and the agg self-term reads the SBUF slice directly.
            XB = 20
            tchunks = [[None] * cfg.NWC for _ in range(2)]
            for w in range(NW):
                if w % XB == 0:
                    whi = min(w + XB, NW)
                    xk_blk = xp.tile([128, whi - w, 128], bf, tag="xk",
                                     name=f"xk{w}")
                    nc.scalar.dma_start(xk_blk[:], P["xc"][:, w:whi, :])
                k, r = w // WPC, w % WPC
                if r == 0:
                    tchunks[0][k] = fp.tile([128, WPC, HID], bf, tag="tch",
                                            name=f"tch_0_{k}")
                ps = psa.tile([128, HID], f32, space="PSUM", tag="a")
                nc.tensor.matmul(out=ps[:], lhsT=xk_blk[:, w % XB, :],
                                 rhs=W1_sb[:], start=True, stop=True)
                nc.vector.tensor_tensor(
                    out=tchunks[0][k][:, r, :], in0=ps[:],
                    in1=invs_sb[:, w:w + 1].to_broadcast([128, HID]),
                    op=mybir.AluOpType.mult)
                if r == WPC - 1:
                    nc.scalar.dma_start(
                        tbl_own[0][k][:].rearrange("(w n) c -> n w c", n=128),
                        tchunks[0][k][:])
                    all_gather(0, k)

            ps_pool = psg.tile([128, G], f32, space="PSUM", tag="g")

            # ---- aggregation sweep (shared for both layers)
            def finish_window(layer, w, ps_w):
                w0 = w * 128
                nc.vector.tensor_tensor(out=ps_w[:], in0=ps_w[:],
                                        in1=invb_sb[:, w0:w0 + 128],
                                        op=mybir.AluOpType.mult)
                if layer == 0:
                    h1w = ep.tile([128, 128], bf, tag="h1w")
                    nc.scalar.activation(h1w[:], ps_w[:],
                                         mybir.ActivationFunctionType.Relu,
                                         bias=b1_sb[:, 0:1])
                    ps2 = psa.tile([128, HID], f32, space="PSUM", tag="a")
                    nc.tensor.matmul(out=ps2[:], lhsT=h1w[:], rhs=W2_sb[:],
                                     start=True, stop=True)
                    k, r = w // WPC, w % WPC
                    if tchunks[1][k] is None:
                        tchunks[1][k] = fp.tile([128, WPC, HID], bf, tag="tch",
                                                name=f"tch_1_{k}")
                    nc.vector.tensor_tensor(
                        out=tchunks[1][k][:, r, :], in0=ps2[:],
                        in1=invs_sb[:, w:w + 1].to_broadcast([128, HID]),
                        op=mybir.AluOpType.mult)
                    tch_done[k] += 1
                    if tch_done[k] == WPC:
                        nc.scalar.dma_start(
                            tbl_own[1][k][:].rearrange("(w n) c -> n w c", n=128),
                            tchunks[1][k][:])
                        all_gather(1, k)
                else:
                    h2w = ep.tile([128, 128], bf, tag="h2w")
                    nc.scalar.activation(h2w[:], ps_w[:],
                                         mybir.ActivationFunctionType.Relu,
                                         bias=b2_sb[:, 0:1])
                    pst = psa.tile([128, 128], f32, space="PSUM", tag="a")
                    nc.tensor.matmul(out=pst[:], lhsT=h2w[:], rhs=identb[:],
                                     start=True, stop=True)
                    h2t = ep.tile([128, 128], bf, tag="h2t")
                    nc.vector.tensor_copy(out=h2t[:], in_=pst[:])
                    sg = sp.tile([128, G], bf, tag="sg")
                    nc.vector.tensor_tensor(
                        out=sg[:],
                        in0=batch_sb[:, w:w + 1].to_broadcast([128, G]),
                        in1=iotag_sb[:],
                        op=mybir.AluOpType.is_equal)
                    nc.tensor.matmul(out=ps_pool[:], lhsT=h2t[:], rhs=sg[:],
                                     start=(w == 0), stop=(w == NW - 1),
                                     skip_group_check=True)

            def agg_layer(layer):
                # Gathers are emitted R supertiles ahead, and the leading
                # R supertiles' gathers are emitted BLOCK-major so the blocks
                # whose AllGather chunk lands first saturate their queues
                # during the staircase (the Pool exec queue holds only 4
                # outstanding DMA instructions, so emit order matters).
                R = 2
                m_all = {}

                def emit_gather(st, b):
                    off = int(meta["seg_off"][st, b])
                    ln = int(meta["seg_len"][st, b])
                    if ln == 0:
                        return
                    mt = mp.tile([128, ln // 128, 128], bf, tag="mtile",
                                 name=f"m_{layer}_{st}_{b}")
                    m_all[(st, b)] = mt
                    if no_gather:
                        nc.gpsimd.memset(mt[:], 0.5)
                    else:
                        nc.gpsimd.dma_gather(
                            mt[:], tbl_full[layer][b][:],
                            idx_sb[:, off // 16: off // 16 + ln // 16],
                            ln, ln, HID,
                            single_packet=False, queue_num=b % 4)

                for b in range(NBLK):
                    for st in range(min(R, NST)):
                        emit_gather(st, b)
                pj = 0
                for st in range(NST):
                    if st + R < NST:
                        for b in range(NBLK):
                            emit_gather(st + R, b)
                    m_tiles = {b: m_all[(st, b)] for b in range(NBLK)
                               if (st, b) in m_all}
                    for (w, lst) in meta["proc"][st]:
                        gl = len(lst)
                        sq = sp.tile([128, gl, 128], bf, tag="s",
                                     name=f"s_{layer}_{w}")
                        nc.vector.tensor_tensor(
                            out=sq[:],
                            in0=dst_sb[:, pj:pj + gl].to_broadcast([128, gl, 128]),
                            in1=iota_sb[:, :gl * 128].rearrange(
                                "p (c j) -> p c j", j=128),
                            op=mybir.AluOpType.is_equal)
                        ps_w = psw.tile([128, 128], f32, space="PSUM",
                                        tag="win", name=f"win_{layer}_{w}")
                        for i, (b, col) in enumerate(lst):
                            nc.tensor.matmul(out=ps_w[:],
                                             lhsT=m_tiles[b][:, col, :],
                                             rhs=sq[:, i, :],
                                             start=(i == 0), stop=False)
                        # self term: += transpose(own table window), read
                        # straight from the resident SBUF chunk tile
                        k, r = w // WPC, w % WPC
                        nc.tensor.matmul(out=ps_w[:],
                                         lhsT=tchunks[layer][k][:, r, :],
                                         rhs=identb[:],
                                         start=False, stop=True)
                        finish_window(layer, w, ps_w)
                        pj += gl

            tch_done = [0] * cfg.NWC
            agg_layer(0)
            agg_layer(1)

            # ---- pooled mean + head (replicated on every core)
            pooled_l = ep.tile([128, G], f32, tag="pool")
            nc.vector.tensor_copy(out=pooled_l[:], in_=ps_pool[:])
            ar_in = dp.tile([128, G], f32, tag="arin")
            ar_out = dp.tile([128, G], f32, addr_space="Shared", tag="arout")
            nc.sync.dma_start(ar_in[:], pooled_l[:])
            if no_coll:
                nc.sync.dma_start(ar_out[:], ar_in[:])
            else:
                nc.gpsimd.collective_compute(
                    "AllReduce", mybir.AluOpType.add,
                    ins=[ar_in.opt()], outs=[ar_out.opt()],
                    replica_groups=replica)
            pooled = ep.tile([128, G], f32, tag="pool")
            nc.sync.dma_start(pooled[:], ar_out[:])

            psc = psg.tile([128, G], f32, space="PSUM", tag="g")
            nc.tensor.matmul(out=psc[:], lhsT=ones1_sb[:], rhs=cnt_sb[:],
                             start=True, stop=True)
            cntb = ep.tile([128, G], f32, tag="pool")
            nc.vector.tensor_scalar_max(out=cntb[:], in0=psc[:], scalar1=1.0)
            invc = ep.tile([128, G], f32, tag="pool")
            nc.vector.reciprocal(invc[:], cntb[:])
            pmean = ep.tile([128, G], f32, tag="pool")
            nc.vector.tensor_tensor(out=pmean[:], in0=pooled[:], in1=invc[:],
                                    op=mybir.AluOpType.mult)

            psh1 = psg.tile([128, G], f32, space="PSUM", tag="g")
            nc.tensor.matmul(out=psh1[:], lhsT=Wh1_sb[:], rhs=pmean[:],
                             start=True, stop=True)
            relu1 = ep.tile([128, G], f32, tag="pool")
            nc.scalar.activation(relu1[:], psh1[:],
                                 mybir.ActivationFunctionType.Relu,
                                 bias=bh1_sb[:, 0:1])
            psh2 = psg.tile([cfg.OUT, G], f32, space="PSUM", tag="g")
            nc.tensor.matmul(out=psh2[:], lhsT=Wh2_sb[:], rhs=relu1[:],
                             start=True, stop=True)
            out_sb = ep.tile([cfg.OUT, G], f32, tag="out")
            nc.vector.tensor_scalar_add(out=out_sb[:], in0=psh2[:],
                                        scalar1=bh2_sb[:, 0:1])
            nc.sync.dma_start(t_out[:], out_sb[:])

    nc.compile()
    return nc


# ----------------------------------------------------------------- entry
def _run(inputs, cfg=CFG, trace=False):
    in_maps, meta = _host_prep(cfg, inputs["x"], inputs["edge_index"],
                               inputs["batch"])
    _add_weights(cfg, in_maps,
                 inputs["W1"], inputs["b1"], inputs["W2"], inputs["b2"],
                 inputs["Wh1"], inputs["bh1"], inputs["Wh2"], inputs["bh2"])
    nc = _build(cfg, meta)
    res = run_bass_kernel_spmd(nc, in_maps, list(range(cfg.C)), trace=trace)
    out = np.ascontiguousarray(np.asarray(res.results[0]["out"]).T)
    return out, res


def kernel(**inputs) -> np.ndarray:
    out, _ = _run(inputs, CFG, trace=False)
    return out
